# revision 15
# baseline (speedup 1.0000x reference)
"""Trainium2 Bass kernel for BackgroundForegroundNeRF (dense per-point MLPs + blend).

Strategy: pure data-parallel over 8 NeuronCores (131072 points each).
Host-side prep: x columns are reordered/padded to 128 (pts at 0:71, views at
96:123) so every on-chip partition access is 32-aligned, and the 14 tiny
weight matrices are packed into 7 combined lhsT layouts (bg+fg nets run
together, block-diagonal / column-packed).

On-chip, channel-major ([channels, points]) per 512-point block:
  - PE transposes the DMA'd point-major x chunks into channel-major PSUM
  - 7 packed float32r matmuls (full fp32 storage, ~tf32 multiply precision)
  - relu / softplus(exp+ln) / copies ride the mandatory PSUM->SBUF crossings
  - a tiny PE transpose of the result channels back to point-major lets the
    sigma blend use per-point free-dim broadcasts, and makes the output DMA
    contiguous.
"""

import numpy as np

N_CORES = 8
NPTS = 1 << 20
PER_CORE = NPTS // N_CORES          # 131072
STEP = 8192                         # points per For_i iteration
BLOCK = 512                         # points per matmul pipeline block
CHUNK = 128                         # points per PE transpose
BLOCKS_PER_STEP = STEP // BLOCK     # 16
CHUNKS_PER_STEP = STEP // CHUNK     # 64
N_STEPS = PER_CORE // STEP          # 16
XCOLS = 128                         # padded x width (pts 0:71, views 96:123)
STAGGERED_RESET = True


def _pack_weights(inp):
    """Pack weights into combined lhsT layouts (matmul: out = lhsT.T @ rhs)."""
    f = np.float32
    bg_s0, bg_s1, bg_s2 = inp["bg_s0"], inp["bg_s1"], inp["bg_s2"]
    fg_s0, fg_s1, fg_s2 = inp["fg_s0"], inp["fg_s1"], inp["fg_s2"]
    bg_c0, bg_c1, bg_c2, bg_c3 = inp["bg_c0"], inp["bg_c1"], inp["bg_c2"], inp["bg_c3"]
    fg_c0, fg_c1, fg_c2, fg_c3 = inp["fg_c0"], inp["fg_c1"], inp["fg_c2"], inp["fg_c3"]

    w1 = np.zeros((71, 128), f)
    w1[0:63, 0:64] = bg_s0.T          # bg uses xyz channels 0:63 only
    w1[0:71, 64:128] = fg_s0.T

    w2 = np.zeros((128, 128), f)
    w2[0:64, 0:64] = bg_s1.T
    w2[64:128, 64:128] = fg_s1.T

    # h3 rows: 0 bg_sigma, 1 fg_unc, 2 fg_sigma, 32:47 bg_geo, 47:62 fg_geo
    w3 = np.zeros((128, 62), f)
    w3[0:64, 0] = bg_s2[0]
    w3[64:128, 1] = fg_s2[1]
    w3[64:128, 2] = fg_s2[0]
    w3[0:64, 32:47] = bg_s2[2:17].T
    w3[64:128, 47:62] = fg_s2[2:17].T

    # ci rows: 0:27 views, 32:47 bg_geo, 47:62 fg_geo
    w4 = np.zeros((62, 128), f)
    w4[0:27, 0:64] = bg_c0[:, 0:27].T
    w4[32:47, 0:64] = bg_c0[:, 27:42].T
    w4[0:27, 64:128] = fg_c0[:, 0:27].T
    w4[47:62, 64:128] = fg_c0[:, 27:42].T

    w5 = np.zeros((128, 128), f)
    w5[0:64, 0:64] = bg_c1.T
    w5[64:128, 64:128] = fg_c1.T

    w6 = np.zeros((128, 128), f)
    w6[0:64, 0:64] = bg_c2.T
    w6[64:128, 64:128] = fg_c2.T

    w7 = np.zeros((128, 6), f)
    w7[0:64, 0:3] = bg_c3.T
    w7[64:128, 3:6] = fg_c3.T

    ident = np.eye(128, dtype=f)
    return {"w1": w1, "w2": w2, "w3": w3, "w4": w4, "w5": w5, "w6": w6,
            "w7": w7, "ident": ident}


_CACHED_NC = {}


def _build_nc(per_core=PER_CORE):
    if per_core in _CACHED_NC:
        return _CACHED_NC[per_core]
    from contextlib import ExitStack

    import concourse.mybir as mybir
    import concourse.tile as tile
    from concourse import bacc
    from concourse.bass import ds

    f32 = mybir.dt.float32
    f32r = mybir.dt.float32r
    AF = mybir.ActivationFunctionType
    ALU = mybir.AluOpType

    nc = bacc.Bacc("TRN2", target_bir_lowering=False, debug=False, num_devices=N_CORES)

    x = nc.dram_tensor("x", [per_core, XCOLS], f32r, kind="ExternalInput").ap()
    wshapes = {"w1": [71, 128], "w2": [128, 128], "w3": [128, 62],
               "w4": [62, 128], "w5": [128, 128], "w6": [128, 128],
               "w7": [128, 6], "ident": [128, 128]}
    wdram = {k: nc.dram_tensor(k, s, f32r, kind="ExternalInput").ap()
             for k, s in wshapes.items()}
    out = nc.dram_tensor("out", [per_core, 6], f32, kind="ExternalOutput").ap()

    with tile.TileContext(nc) as tc, ExitStack() as ctx:
        wpool = ctx.enter_context(tc.tile_pool(name="w", bufs=1))
        wt = {}
        for k, s in wshapes.items():
            wt[k] = wpool.tile(s, f32r, tag=k, name=k)
            nc.sync.dma_start(wt[k][:], wdram[k][:])

        xpool = ctx.enter_context(tc.tile_pool(name="xraw", bufs=2))
        xapool = ctx.enter_context(tc.tile_pool(name="xa", bufs=2))
        apool = ctx.enter_context(tc.tile_pool(name="act", bufs=3))
        prepool = ctx.enter_context(tc.tile_pool(name="pre", bufs=2))
        bpool = ctx.enter_context(tc.tile_pool(name="blend", bufs=2))
        ps_x = ctx.enter_context(tc.tile_pool(name="ps_x", bufs=2, space="PSUM"))
        ps_h = ctx.enter_context(tc.tile_pool(name="ps_h", bufs=3, space="PSUM"))
        ps_c = ctx.enter_context(tc.tile_pool(name="ps_c", bufs=1, space="PSUM"))
        ps_pm = ctx.enter_context(tc.tile_pool(name="ps_pm", bufs=2, space="PSUM"))

        def do_block(xraw, b, base):
            # ---- transpose x chunks to channel-major ----
            xTp = ps_x.tile([128, BLOCK], f32r, tag="xtp")
            for c in range(4):
                nc.tensor.transpose(
                    xTp[:, CHUNK * c:CHUNK * (c + 1)],
                    xraw[:, 4 * b + c, :],
                    wt["ident"][:],
                )
            xa = xapool.tile([71, BLOCK], f32r, tag="xa")
            nc.scalar.copy(xa[0:71, :], xTp[0:71, :])                    # ACT
            # color-net input: rows 0:27 views, 32:47 bg_geo, 47:62 fg_geo
            ci = xapool.tile([62, BLOCK], f32r, tag="ci")
            nc.vector.tensor_copy(ci[0:32, :], xTp[96:128, :])           # DVE

            # ---- sigma nets (bg+fg packed) ----
            h1 = ps_h.tile([128, BLOCK], f32, tag="h")
            nc.tensor.matmul(h1[:], wt["w1"][:], xa[0:71, :],
                             start=True, stop=True)
            a1 = apool.tile([128, BLOCK], f32r, tag="a")
            nc.vector.tensor_relu(a1[:], h1[:])                          # DVE

            h2 = ps_h.tile([128, BLOCK], f32, tag="h")
            nc.tensor.matmul(h2[:], wt["w2"][:], a1[:],
                             start=True, stop=True)
            a2 = apool.tile([128, BLOCK], f32r, tag="a")
            nc.scalar.activation(a2[:], h2[:], AF.Relu)                  # ACT

            h3 = ps_h.tile([62, BLOCK], f32, tag="h")
            nc.tensor.matmul(h3[:], wt["w3"][:], a2[:],
                             start=True, stop=True)

            pre = prepool.tile([38, BLOCK], f32r, tag="pre")
            nc.gpsimd.memset(pre[3:32, :], 0.0)                          # POOL
            # softplus(x) = ln(1 + exp(x)); Softplus has no ACT table on gen3
            esp = prepool.tile([3, BLOCK], f32, tag="esp")
            nc.scalar.activation(esp[:], h3[0:3, :], AF.Exp)             # ACT
            nc.scalar.activation(pre[0:3, :], esp[:], AF.Ln, bias=1.0)   # ACT
            nc.vector.tensor_copy(ci[32:62, :], h3[32:62, :])            # DVE geo

            # ---- color nets ----
            h4 = ps_h.tile([128, BLOCK], f32, tag="h")
            nc.tensor.matmul(h4[:], wt["w4"][:], ci[0:62, :],
                             start=True, stop=True)
            a4 = apool.tile([128, BLOCK], f32r, tag="a")
            nc.vector.tensor_relu(a4[:], h4[:])                          # DVE

            h5 = ps_h.tile([128, BLOCK], f32, tag="h")
            nc.tensor.matmul(h5[:], wt["w5"][:], a4[:],
                             start=True, stop=True)
            a5 = apool.tile([128, BLOCK], f32r, tag="a")
            nc.vector.tensor_relu(a5[:], h5[:])                          # DVE

            h6 = ps_h.tile([128, BLOCK], f32, tag="h")
            nc.tensor.matmul(h6[:], wt["w6"][:], a5[:],
                             start=True, stop=True)
            a6 = apool.tile([128, BLOCK], f32r, tag="a")
            nc.vector.tensor_relu(a6[:], h6[:])                          # DVE

            c6 = ps_c.tile([6, BLOCK], f32, tag="c6")
            nc.tensor.matmul(c6[:], wt["w7"][:], a6[:],
                             start=True, stop=True)
            nc.scalar.copy(pre[32:38, :], c6[0:6, :])                    # ACT

            # ---- back to point-major ----
            # pre rows: 0 s_bg, 1 u_fg, 2 s_fg, 32:35 bg_color, 35:38 fg_color
            pm = ps_pm.tile([128, 4, 38], f32r, tag="pm")
            for c in range(4):
                nc.tensor.transpose(
                    pm[:, c, :],
                    pre[:, CHUNK * c:CHUNK * (c + 1)],
                    wt["ident"][0:38, 0:38],
                )
            pms = bpool.tile([128, 4, 38], f32, tag="pms")
            nc.vector.tensor_copy(pms[:], pm[:])                         # DVE

            # ---- blend (per-point, free-dim ops) ----
            o = bpool.tile([128, 4, 6], f32, tag="o")
            # sigma = s_bg + s_fg + 1e-9
            nc.vector.scalar_tensor_tensor(
                o[:, :, 3:4], pms[:, :, 0:1], 1e-9, pms[:, :, 2:3],
                ALU.add, ALU.add)
            rcp = bpool.tile([128, 4, 1], f32, tag="rcp")
            nc.vector.reciprocal(rcp[:], o[:, :, 3:4])                   # DVE
            wb = bpool.tile([128, 4, 1], f32, tag="wb")
            nc.gpsimd.tensor_mul(wb[:], pms[:, :, 0:1], rcp[:])
            wf = bpool.tile([128, 4, 1], f32, tag="wf")
            nc.gpsimd.tensor_mul(wf[:], pms[:, :, 2:3], rcp[:])
            t1 = bpool.tile([128, 4, 3], f32, tag="t1")
            nc.gpsimd.tensor_mul(t1[:], pms[:, :, 32:35],
                                 wb[:].to_broadcast((128, 4, 3)))
            t2 = bpool.tile([128, 4, 3], f32, tag="t2")
            nc.gpsimd.tensor_mul(t2[:], pms[:, :, 35:38],
                                 wf[:].to_broadcast((128, 4, 3)))
            nc.gpsimd.tensor_add(o[:, :, 0:3], t1[:], t2[:])
            nc.gpsimd.tensor_copy(o[:, :, 4:6], pms[:, :, 1:3])

            nc.sync.dma_start(
                out[ds(base + b * BLOCK, BLOCK)].rearrange(
                    "(c p) f -> p c f", p=128),
                o[:])

        with tc.For_i(0, per_core, STEP, staggered_reset=STAGGERED_RESET) as basev:
            xraw = xpool.tile([128, CHUNKS_PER_STEP, XCOLS], f32r, tag="xraw")
            nc.sync.dma_start(
                xraw[:],
                x[ds(basev, STEP)].rearrange("(c p) f -> p c f", p=128))
            for b in range(BLOCKS_PER_STEP):
                do_block(xraw, b, basev)

    nc.compile()
    nc._dram_aps = {"x": x, "out": out, **wdram}
    _CACHED_NC[per_core] = nc
    return nc


def _prep_x(x):
    x = np.asarray(x, dtype=np.float32)
    xp = np.zeros((x.shape[0], XCOLS), np.float32)
    xp[:, 0:71] = x[:, 0:71]
    xp[:, 96:123] = x[:, 71:98]
    return xp


def kernel(**inputs):
    from concourse.bass_utils import run_bass_kernel_spmd

    nc = _build_nc()
    packed = _pack_weights(inputs)
    xp = _prep_x(inputs["x"])
    in_maps = []
    for c in range(N_CORES):
        m = {"x": xp[c * PER_CORE:(c + 1) * PER_CORE]}
        m.update(packed)
        in_maps.append(m)
    res = run_bass_kernel_spmd(nc, in_maps, core_ids=list(range(N_CORES)))
    return np.concatenate([r["out"] for r in res.results], axis=0)


# revision 24
# speedup vs baseline: 49.2952x; 49.2952x over previous
"""Trainium2 Bass kernel for BackgroundForegroundNeRF (dense per-point MLPs + blend).

Strategy: pure data-parallel over 8 NeuronCores (131072 points each).
Host-side prep: x columns are reordered/padded to 128 (pts at 0:71, views at
96:123) so every on-chip partition access is 32-aligned, and the 14 tiny
weight matrices are packed into 7 combined lhsT layouts (bg+fg nets run
together, block-diagonal / column-packed).

On-chip, channel-major ([channels, points]) per 512-point block:
  - PE transposes the DMA'd point-major x chunks into channel-major PSUM
  - 7 packed float32r matmuls (full fp32 storage, ~tf32 multiply precision)
  - relu / softplus(exp+ln) / copies ride the mandatory PSUM->SBUF crossings
  - a tiny PE transpose of the result channels back to point-major lets the
    sigma blend use per-point free-dim broadcasts, and makes the output DMA
    contiguous.
"""

import numpy as np

N_CORES = 8
NPTS = 1 << 20
PER_CORE = NPTS // N_CORES          # 131072
STEP = 8192                         # points per For_i iteration
BLOCK = 512                         # points per matmul pipeline block
CHUNK = 128                         # points per PE transpose
BLOCKS_PER_STEP = STEP // BLOCK     # 16
CHUNKS_PER_STEP = STEP // CHUNK     # 64
N_STEPS = PER_CORE // STEP          # 16
XCOLS = 128                         # padded x width (pts 0:71, views 96:123)
INTERLEAVE = 2                      # blocks emitted op-interleaved
STAGGERED_RESET = True


def _pack_weights(inp):
    """Pack weights into combined lhsT layouts (matmul: out = lhsT.T @ rhs)."""
    f = np.float32
    bg_s0, bg_s1, bg_s2 = inp["bg_s0"], inp["bg_s1"], inp["bg_s2"]
    fg_s0, fg_s1, fg_s2 = inp["fg_s0"], inp["fg_s1"], inp["fg_s2"]
    bg_c0, bg_c1, bg_c2, bg_c3 = inp["bg_c0"], inp["bg_c1"], inp["bg_c2"], inp["bg_c3"]
    fg_c0, fg_c1, fg_c2, fg_c3 = inp["fg_c0"], inp["fg_c1"], inp["fg_c2"], inp["fg_c3"]

    w1 = np.zeros((71, 128), f)
    w1[0:63, 0:64] = bg_s0.T          # bg uses xyz channels 0:63 only
    w1[0:71, 64:128] = fg_s0.T

    w2 = np.zeros((128, 128), f)
    w2[0:64, 0:64] = bg_s1.T
    w2[64:128, 64:128] = fg_s1.T

    # h3 rows: 0 bg_sigma, 1 fg_unc, 2 fg_sigma, 32:47 bg_geo, 47:62 fg_geo
    w3 = np.zeros((128, 62), f)
    w3[0:64, 0] = bg_s2[0]
    w3[64:128, 1] = fg_s2[1]
    w3[64:128, 2] = fg_s2[0]
    w3[0:64, 32:47] = bg_s2[2:17].T
    w3[64:128, 47:62] = fg_s2[2:17].T

    # ci rows: 0:27 views, 32:47 bg_geo, 47:62 fg_geo
    w4 = np.zeros((62, 128), f)
    w4[0:27, 0:64] = bg_c0[:, 0:27].T
    w4[32:47, 0:64] = bg_c0[:, 27:42].T
    w4[0:27, 64:128] = fg_c0[:, 0:27].T
    w4[47:62, 64:128] = fg_c0[:, 27:42].T

    w5 = np.zeros((128, 128), f)
    w5[0:64, 0:64] = bg_c1.T
    w5[64:128, 64:128] = fg_c1.T

    w6 = np.zeros((128, 128), f)
    w6[0:64, 0:64] = bg_c2.T
    w6[64:128, 64:128] = fg_c2.T

    w7 = np.zeros((128, 6), f)
    w7[0:64, 0:3] = bg_c3.T
    w7[64:128, 3:6] = fg_c3.T

    ident = np.eye(128, dtype=f)
    return {"w1": w1, "w2": w2, "w3": w3, "w4": w4, "w5": w5, "w6": w6,
            "w7": w7, "ident": ident}


_CACHED_NC = {}


def _build_nc(per_core=PER_CORE):
    if per_core in _CACHED_NC:
        return _CACHED_NC[per_core]
    from contextlib import ExitStack

    import concourse.mybir as mybir
    import concourse.tile as tile
    from concourse import bacc
    from concourse.bass import ds

    f32 = mybir.dt.float32
    f32r = mybir.dt.float32r
    AF = mybir.ActivationFunctionType
    ALU = mybir.AluOpType

    nc = bacc.Bacc("TRN2", target_bir_lowering=False, debug=False, num_devices=N_CORES)

    x = nc.dram_tensor("x", [per_core, XCOLS], f32r, kind="ExternalInput").ap()
    wshapes = {"w1": [71, 128], "w2": [128, 128], "w3": [128, 62],
               "w4": [62, 128], "w5": [128, 128], "w6": [128, 128],
               "w7": [128, 6], "ident": [128, 128]}
    wdram = {k: nc.dram_tensor(k, s, f32r, kind="ExternalInput").ap()
             for k, s in wshapes.items()}
    out = nc.dram_tensor("out", [per_core, 6], f32, kind="ExternalOutput").ap()

    with tile.TileContext(nc) as tc, ExitStack() as ctx:
        wpool = ctx.enter_context(tc.tile_pool(name="w", bufs=1))
        wt = {}
        for k, s in wshapes.items():
            wt[k] = wpool.tile(s, f32r, tag=k, name=k)
            nc.sync.dma_start(wt[k][:], wdram[k][:])

        xpool = ctx.enter_context(tc.tile_pool(name="xraw", bufs=3))
        xapool = ctx.enter_context(tc.tile_pool(name="xa", bufs=5))
        apool = ctx.enter_context(tc.tile_pool(name="act", bufs=6))
        prepool = ctx.enter_context(tc.tile_pool(name="pre", bufs=4))
        bpool = ctx.enter_context(tc.tile_pool(name="blend", bufs=4))
        ps_x = ctx.enter_context(tc.tile_pool(name="ps_x", bufs=2, space="PSUM"))
        ps_h = ctx.enter_context(tc.tile_pool(name="ps_h", bufs=3, space="PSUM"))
        ps_c = ctx.enter_context(tc.tile_pool(name="ps_c", bufs=1, space="PSUM"))
        ps_pm = ctx.enter_context(tc.tile_pool(name="ps_pm", bufs=2, space="PSUM"))

        def do_block(xraw, b, base, pm, boff):
            # generator: yields between ops so two blocks can interleave
            # ---- transpose x chunks to channel-major ----
            xTp = ps_x.tile([128, BLOCK], f32r, tag="xtp")
            for c in range(4):
                nc.tensor.transpose(
                    xTp[:, CHUNK * c:CHUNK * (c + 1)],
                    xraw[:, 4 * b + c, :],
                    wt["ident"][:],
                )
            yield
            xa = xapool.tile([71, BLOCK], f32r, tag="xa")
            nc.scalar.copy(xa[0:71, :], xTp[0:71, :])                    # ACT
            # color-net input: rows 0:27 views, 32:47 bg_geo, 47:62 fg_geo
            ci = xapool.tile([62, BLOCK], f32r, tag="ci")
            nc.vector.tensor_copy(ci[0:32, :], xTp[96:128, :])           # DVE

            yield
            # ---- sigma nets (bg+fg packed) ----
            h1 = ps_h.tile([128, BLOCK], f32, tag="h")
            nc.tensor.matmul(h1[:], wt["w1"][:], xa[0:71, :],
                             start=True, stop=True)
            a1 = apool.tile([128, BLOCK], f32r, tag="a")
            nc.vector.tensor_relu(a1[:], h1[:])                          # DVE

            yield
            h2 = ps_h.tile([128, BLOCK], f32, tag="h")
            nc.tensor.matmul(h2[:], wt["w2"][:], a1[:],
                             start=True, stop=True)
            a2 = apool.tile([128, BLOCK], f32r, tag="a")
            nc.scalar.activation(a2[:], h2[:], AF.Relu)                  # ACT

            yield
            h3 = ps_h.tile([62, BLOCK], f32, tag="h")
            nc.tensor.matmul(h3[:], wt["w3"][:], a2[:],
                             start=True, stop=True)

            yield
            pre = prepool.tile([38, BLOCK], f32r, tag="pre")
            nc.gpsimd.memset(pre[0:32, :].bitcast(mybir.dt.uint32), 0)   # POOL
            # softplus(x) = ln(1 + exp(x)); Softplus has no ACT table on gen3
            esp = prepool.tile([3, BLOCK], f32, tag="esp")
            nc.scalar.activation(esp[:], h3[0:3, :], AF.Exp)             # ACT
            nc.scalar.activation(pre[0:3, :], esp[:], AF.Ln, bias=1.0)   # ACT
            nc.vector.tensor_copy(ci[32:62, :], h3[32:62, :])            # DVE geo

            yield
            # ---- color nets ----
            h4 = ps_h.tile([128, BLOCK], f32, tag="h")
            nc.tensor.matmul(h4[:], wt["w4"][:], ci[0:62, :],
                             start=True, stop=True)
            a4 = apool.tile([128, BLOCK], f32r, tag="a")
            nc.vector.tensor_relu(a4[:], h4[:])                          # DVE

            yield
            h5 = ps_h.tile([128, BLOCK], f32, tag="h")
            nc.tensor.matmul(h5[:], wt["w5"][:], a4[:],
                             start=True, stop=True)
            a5 = apool.tile([128, BLOCK], f32r, tag="a")
            nc.vector.tensor_relu(a5[:], h5[:])                          # DVE

            yield
            h6 = ps_h.tile([128, BLOCK], f32, tag="h")
            nc.tensor.matmul(h6[:], wt["w6"][:], a5[:],
                             start=True, stop=True)
            a6 = apool.tile([128, BLOCK], f32r, tag="a")
            nc.vector.tensor_relu(a6[:], h6[:])                          # DVE

            yield
            c6 = ps_c.tile([6, BLOCK], f32, tag="c6")
            nc.tensor.matmul(c6[:], wt["w7"][:], a6[:],
                             start=True, stop=True)
            nc.scalar.copy(pre[32:38, :], c6[0:6, :])                    # ACT

            yield
            # ---- back to point-major ----
            # pre rows: 0 s_bg, 1 u_fg, 2 s_fg, 32:35 bg_color, 35:38 fg_color
            for c in range(4):
                nc.tensor.transpose(
                    pm[:, 4 * boff + c, :],
                    pre[:, CHUNK * c:CHUNK * (c + 1)],
                    wt["ident"][0:38, 0:38],
                )

        with tc.For_i(0, per_core, STEP, staggered_reset=STAGGERED_RESET) as basev:
            xraw = xpool.tile([128, CHUNKS_PER_STEP, XCOLS], f32r, tag="xraw")
            nc.sync.dma_start(
                xraw[:],
                x[ds(basev, STEP)].rearrange("(c p) f -> p c f", p=128))
            for pb in range(BLOCKS_PER_STEP // INTERLEAVE):
                pm = ps_pm.tile([128, 4 * INTERLEAVE, 38], f32r, tag="pm")
                live = [do_block(xraw, INTERLEAVE * pb + j, basev, pm, j)
                        for j in range(INTERLEAVE)]
                while live:
                    for g in list(live):
                        try:
                            next(g)
                        except StopIteration:
                            live.remove(g)

                # ---- pair-level blend over 4*INTERLEAVE point-major chunks
                nch = 4 * INTERLEAVE
                pms = bpool.tile([128, nch, 38], f32, tag="pms")
                nc.vector.tensor_copy(pms[:], pm[:])                     # DVE
                o = bpool.tile([128, nch, 6], f32, tag="o")
                nc.vector.scalar_tensor_tensor(
                    o[:, :, 3:4], pms[:, :, 0:1], 1e-9, pms[:, :, 2:3],
                    ALU.add, ALU.add)
                rcp = bpool.tile([128, nch, 1], f32, tag="rcp")
                nc.vector.reciprocal(rcp[:], o[:, :, 3:4])               # DVE
                wb = bpool.tile([128, nch, 1], f32, tag="wb")
                nc.gpsimd.tensor_mul(wb[:], pms[:, :, 0:1], rcp[:])
                wf = bpool.tile([128, nch, 1], f32, tag="wf")
                nc.gpsimd.tensor_mul(wf[:], pms[:, :, 2:3], rcp[:])
                t1 = bpool.tile([128, nch, 3], f32, tag="t1")
                nc.gpsimd.tensor_mul(t1[:], pms[:, :, 32:35],
                                     wb[:].to_broadcast((128, nch, 3)))
                t2 = bpool.tile([128, nch, 3], f32, tag="t2")
                nc.gpsimd.tensor_mul(t2[:], pms[:, :, 35:38],
                                     wf[:].to_broadcast((128, nch, 3)))
                nc.gpsimd.tensor_add(o[:, :, 0:3], t1[:], t2[:])
                nc.gpsimd.tensor_copy(o[:, :, 4:6], pms[:, :, 1:3])
                nc.sync.dma_start(
                    out[ds(basev + pb * INTERLEAVE * BLOCK,
                           INTERLEAVE * BLOCK)].rearrange(
                        "(c p) f -> p c f", p=128),
                    o[:])

    nc.compile()
    nc._dram_aps = {"x": x, "out": out, **wdram}
    _CACHED_NC[per_core] = nc
    return nc


def _prep_x(x):
    x = np.asarray(x, dtype=np.float32)
    xp = np.zeros((x.shape[0], XCOLS), np.float32)
    xp[:, 0:71] = x[:, 0:71]
    xp[:, 96:123] = x[:, 71:98]
    return xp


def kernel(**inputs):
    from concourse.bass_utils import run_bass_kernel_spmd

    nc = _build_nc()
    packed = _pack_weights(inputs)
    xp = _prep_x(inputs["x"])
    in_maps = []
    for c in range(N_CORES):
        m = {"x": xp[c * PER_CORE:(c + 1) * PER_CORE]}
        m.update(packed)
        in_maps.append(m)
    res = run_bass_kernel_spmd(nc, in_maps, core_ids=list(range(N_CORES)))
    return np.concatenate([r["out"] for r in res.results], axis=0)


# revision 32
# speedup vs baseline: 50.5908x; 1.0263x over previous
"""Trainium2 Bass kernel for BackgroundForegroundNeRF (dense per-point MLPs + blend).

Strategy: pure data-parallel over 8 NeuronCores (131072 points each).
Host-side prep: x columns are reordered/padded to 128 (pts at 0:71, views at
96:123) so every on-chip partition access is 32-aligned, and the 14 tiny
weight matrices are packed into 7 combined lhsT layouts (bg+fg nets run
together, block-diagonal / column-packed).

On-chip, channel-major ([channels, points]) per 512-point block:
  - PE transposes the DMA'd point-major x chunks into channel-major PSUM
  - 7 packed float32r matmuls (full fp32 storage, ~tf32 multiply precision)
  - relu / softplus(exp+ln) / copies ride the mandatory PSUM->SBUF crossings
  - a tiny PE transpose of the result channels back to point-major lets the
    sigma blend use per-point free-dim broadcasts, and makes the output DMA
    contiguous.
"""

import numpy as np

N_CORES = 8
NPTS = 1 << 20
PER_CORE = NPTS // N_CORES          # 131072
STEP = 8192                         # points per For_i iteration
BLOCK = 512                         # points per matmul pipeline block
CHUNK = 128                         # points per PE transpose
BLOCKS_PER_STEP = STEP // BLOCK     # 16
CHUNKS_PER_STEP = STEP // CHUNK     # 64
N_STEPS = PER_CORE // STEP          # 16
XCOLS = 128                         # padded x width (pts 0:71, views 96:123)
INTERLEAVE = 2                      # blocks emitted op-interleaved
STAGGERED_RESET = True


def _pack_weights(inp):
    """Pack weights into combined lhsT layouts (matmul: out = lhsT.T @ rhs)."""
    f = np.float32
    bg_s0, bg_s1, bg_s2 = inp["bg_s0"], inp["bg_s1"], inp["bg_s2"]
    fg_s0, fg_s1, fg_s2 = inp["fg_s0"], inp["fg_s1"], inp["fg_s2"]
    bg_c0, bg_c1, bg_c2, bg_c3 = inp["bg_c0"], inp["bg_c1"], inp["bg_c2"], inp["bg_c3"]
    fg_c0, fg_c1, fg_c2, fg_c3 = inp["fg_c0"], inp["fg_c1"], inp["fg_c2"], inp["fg_c3"]

    w1 = np.zeros((71, 128), f)
    w1[0:63, 0:64] = bg_s0.T          # bg uses xyz channels 0:63 only
    w1[0:71, 64:128] = fg_s0.T

    w2 = np.zeros((128, 128), f)
    w2[0:64, 0:64] = bg_s1.T
    w2[64:128, 64:128] = fg_s1.T

    # h3 rows: 0 bg_sigma, 1 fg_unc, 2 fg_sigma, 32:47 bg_geo, 47:62 fg_geo
    w3 = np.zeros((128, 62), f)
    w3[0:64, 0] = bg_s2[0]
    w3[64:128, 1] = fg_s2[1]
    w3[64:128, 2] = fg_s2[0]
    w3[0:64, 32:47] = bg_s2[2:17].T
    w3[64:128, 47:62] = fg_s2[2:17].T

    # ci rows: 0:27 views, 32:47 bg_geo, 47:62 fg_geo
    w4 = np.zeros((62, 128), f)
    w4[0:27, 0:64] = bg_c0[:, 0:27].T
    w4[32:47, 0:64] = bg_c0[:, 27:42].T
    w4[0:27, 64:128] = fg_c0[:, 0:27].T
    w4[47:62, 64:128] = fg_c0[:, 27:42].T

    w5 = np.zeros((128, 128), f)
    w5[0:64, 0:64] = bg_c1.T
    w5[64:128, 64:128] = fg_c1.T

    w6 = np.zeros((128, 128), f)
    w6[0:64, 0:64] = bg_c2.T
    w6[64:128, 64:128] = fg_c2.T

    w7 = np.zeros((128, 6), f)
    w7[0:64, 0:3] = bg_c3.T
    w7[64:128, 3:6] = fg_c3.T

    ident = np.eye(128, dtype=f)
    return {"w1": w1, "w2": w2, "w3": w3, "w4": w4, "w5": w5, "w6": w6,
            "w7": w7, "ident": ident}


_CACHED_NC = {}


def _build_nc(per_core=PER_CORE):
    if per_core in _CACHED_NC:
        return _CACHED_NC[per_core]
    from contextlib import ExitStack

    import concourse.mybir as mybir
    import concourse.tile as tile
    from concourse import bacc
    from concourse.bass import ds

    f32 = mybir.dt.float32
    f32r = mybir.dt.float32r
    AF = mybir.ActivationFunctionType
    ALU = mybir.AluOpType

    nc = bacc.Bacc("TRN2", target_bir_lowering=False, debug=False, num_devices=N_CORES)

    x = nc.dram_tensor("x", [per_core, XCOLS], f32r, kind="ExternalInput").ap()
    wshapes = {"w1": [71, 128], "w2": [128, 128], "w3": [128, 62],
               "w4": [62, 128], "w5": [128, 128], "w6": [128, 128],
               "w7": [128, 6], "ident": [128, 128]}
    wdram = {k: nc.dram_tensor(k, s, f32r, kind="ExternalInput").ap()
             for k, s in wshapes.items()}
    out = nc.dram_tensor("out", [per_core, 6], f32, kind="ExternalOutput").ap()

    with tile.TileContext(nc) as tc, ExitStack() as ctx:
        wpool = ctx.enter_context(tc.tile_pool(name="w", bufs=1))
        wt = {}
        for k, s in wshapes.items():
            wt[k] = wpool.tile(s, f32r, tag=k, name=k)
            nc.sync.dma_start(wt[k][:], wdram[k][:])

        xpool = ctx.enter_context(tc.tile_pool(name="xraw", bufs=3))
        xapool = ctx.enter_context(tc.tile_pool(name="xa", bufs=5))
        apool = ctx.enter_context(tc.tile_pool(name="act", bufs=6))
        prepool = ctx.enter_context(tc.tile_pool(name="pre", bufs=4))
        bpool = ctx.enter_context(tc.tile_pool(name="blend", bufs=4))
        ps_x = ctx.enter_context(tc.tile_pool(name="ps_x", bufs=2, space="PSUM"))
        ps_h = ctx.enter_context(tc.tile_pool(name="ps_h", bufs=4, space="PSUM"))
        ps_pm = ctx.enter_context(tc.tile_pool(name="ps_pm", bufs=2, space="PSUM"))

        def do_block(xraw, b, base, pm, boff):
            # generator: yields between ops so two blocks can interleave
            # ---- transpose x chunks to channel-major ----
            xTp = ps_x.tile([128, BLOCK], f32r, tag="xtp")
            for c in range(4):
                nc.tensor.transpose(
                    xTp[:, CHUNK * c:CHUNK * (c + 1)],
                    xraw[:, 4 * b + c, :],
                    wt["ident"][:],
                )
            yield
            xa = xapool.tile([71, BLOCK], f32r, tag="xa")
            nc.scalar.copy(xa[0:71, :], xTp[0:71, :])                    # ACT
            # color-net input: rows 0:27 views, 32:47 bg_geo, 47:62 fg_geo
            ci = xapool.tile([62, BLOCK], f32r, tag="ci")
            nc.vector.tensor_copy(ci[0:32, :], xTp[96:128, :])           # DVE

            yield
            # ---- sigma nets (bg+fg packed) ----
            h1 = ps_h.tile([128, BLOCK], f32, tag="h")
            nc.tensor.matmul(h1[:], wt["w1"][:], xa[0:71, :],
                             start=True, stop=True)
            a1 = apool.tile([128, BLOCK], f32r, tag="a")
            nc.vector.tensor_relu(a1[:], h1[:])                          # DVE

            yield
            h2 = ps_h.tile([128, BLOCK], f32, tag="h")
            nc.tensor.matmul(h2[:], wt["w2"][:], a1[:],
                             start=True, stop=True)
            a2 = apool.tile([128, BLOCK], f32r, tag="a")
            nc.scalar.activation(a2[:], h2[:], AF.Relu)                  # ACT

            yield
            h3 = ps_h.tile([62, BLOCK], f32, tag="h")
            nc.tensor.matmul(h3[:], wt["w3"][:], a2[:],
                             start=True, stop=True)

            yield
            pre = prepool.tile([4, BLOCK], f32r, tag="pre")
            # softplus(x) = ln(1 + exp(x)); Softplus has no ACT table on gen3
            # (row 3 rides along as softplus(0): fp32r transpose needs even N)
            esp = prepool.tile([4, BLOCK], f32, tag="esp")
            nc.scalar.activation(esp[:], h3[0:4, :], AF.Exp)             # ACT
            nc.scalar.activation(pre[0:4, :], esp[:], AF.Ln, bias=1.0)   # ACT
            nc.vector.tensor_copy(ci[32:62, :], h3[32:62, :])            # DVE geo

            yield
            # ---- color nets ----
            h4 = ps_h.tile([128, BLOCK], f32, tag="h")
            nc.tensor.matmul(h4[:], wt["w4"][:], ci[0:62, :],
                             start=True, stop=True)
            a4 = apool.tile([128, BLOCK], f32r, tag="a")
            nc.vector.tensor_relu(a4[:], h4[:])                          # DVE

            yield
            h5 = ps_h.tile([128, BLOCK], f32, tag="h")
            nc.tensor.matmul(h5[:], wt["w5"][:], a4[:],
                             start=True, stop=True)
            a5 = apool.tile([128, BLOCK], f32r, tag="a")
            nc.vector.tensor_relu(a5[:], h5[:])                          # DVE

            yield
            h6 = ps_h.tile([128, BLOCK], f32, tag="h")
            nc.tensor.matmul(h6[:], wt["w6"][:], a5[:],
                             start=True, stop=True)
            a6 = apool.tile([128, BLOCK], f32r, tag="a")
            nc.vector.tensor_relu(a6[:], h6[:])                          # DVE

            yield
            yield
            # ---- last color layer directly point-major ----
            # out[pt, 0:3] = bg_color, out[pt, 3:6] = fg_color (lhsT = a6 chunk)
            for c in range(4):
                nc.tensor.matmul(pm[:, 4 * boff + c, 4:10],
                                 a6[:, CHUNK * c:CHUNK * (c + 1)].bitcast(f32),
                                 wt["w7"][:].bitcast(f32),
                                 start=True, stop=True)
            # sigmas to point-major: pm cols 0 s_bg, 1 u_fg, 2 s_fg
            for c in range(4):
                nc.tensor.transpose(
                    pm[:, 4 * boff + c, 0:4].bitcast(f32r),
                    pre[:, CHUNK * c:CHUNK * (c + 1)],
                    wt["ident"][0:4, 0:4],
                )

        with tc.For_i(0, per_core, STEP, staggered_reset=STAGGERED_RESET) as basev:
            xraw = xpool.tile([128, CHUNKS_PER_STEP, XCOLS], f32r, tag="xraw")
            nc.sync.dma_start(
                xraw[:],
                x[ds(basev, STEP)].rearrange("(c p) f -> p c f", p=128))
            for pb in range(BLOCKS_PER_STEP // INTERLEAVE):
                pm = ps_pm.tile([128, 4 * INTERLEAVE, 10], f32, tag="pm")
                live = [do_block(xraw, INTERLEAVE * pb + j, basev, pm, j)
                        for j in range(INTERLEAVE)]
                while live:
                    for g in list(live):
                        try:
                            next(g)
                        except StopIteration:
                            live.remove(g)

                # ---- pair-level blend over 4*INTERLEAVE point-major chunks
                nch = 4 * INTERLEAVE
                pms = bpool.tile([128, nch, 10], f32, tag="pms")
                nc.vector.tensor_copy(pms[:], pm[:])                     # DVE
                o = bpool.tile([128, nch, 6], f32, tag="o")
                nc.vector.scalar_tensor_tensor(
                    o[:, :, 3:4], pms[:, :, 0:1], 1e-9, pms[:, :, 2:3],
                    ALU.add, ALU.add)
                rcp = bpool.tile([128, nch, 1], f32, tag="rcp")
                nc.vector.reciprocal(rcp[:], o[:, :, 3:4])               # DVE
                wb = bpool.tile([128, nch, 1], f32, tag="wb")
                nc.gpsimd.tensor_mul(wb[:], pms[:, :, 0:1], rcp[:])
                wf = bpool.tile([128, nch, 1], f32, tag="wf")
                nc.gpsimd.tensor_mul(wf[:], pms[:, :, 2:3], rcp[:])
                t1 = bpool.tile([128, nch, 3], f32, tag="t1")
                nc.gpsimd.tensor_mul(t1[:], pms[:, :, 4:7],
                                     wb[:].to_broadcast((128, nch, 3)))
                t2 = bpool.tile([128, nch, 3], f32, tag="t2")
                nc.gpsimd.tensor_mul(t2[:], pms[:, :, 7:10],
                                     wf[:].to_broadcast((128, nch, 3)))
                nc.gpsimd.tensor_add(o[:, :, 0:3], t1[:], t2[:])
                nc.gpsimd.tensor_copy(o[:, :, 4:6], pms[:, :, 1:3])
                nc.sync.dma_start(
                    out[ds(basev + pb * INTERLEAVE * BLOCK,
                           INTERLEAVE * BLOCK)].rearrange(
                        "(c p) f -> p c f", p=128),
                    o[:])

    nc.compile()
    nc._dram_aps = {"x": x, "out": out, **wdram}
    _CACHED_NC[per_core] = nc
    return nc


def _prep_x(x):
    x = np.asarray(x, dtype=np.float32)
    xp = np.zeros((x.shape[0], XCOLS), np.float32)
    xp[:, 0:71] = x[:, 0:71]
    xp[:, 96:123] = x[:, 71:98]
    return xp


def kernel(**inputs):
    from concourse.bass_utils import run_bass_kernel_spmd

    nc = _build_nc()
    packed = _pack_weights(inputs)
    xp = _prep_x(inputs["x"])
    in_maps = []
    for c in range(N_CORES):
        m = {"x": xp[c * PER_CORE:(c + 1) * PER_CORE]}
        m.update(packed)
        in_maps.append(m)
    res = run_bass_kernel_spmd(nc, in_maps, core_ids=list(range(N_CORES)))
    return np.concatenate([r["out"] for r in res.results], axis=0)
